# revision 6
# baseline (speedup 1.0000x reference)
"""BPCA Unpooling kernel for Trainium2 (8 NeuronCores, data-parallel over batch).

Math per sample s (reference semantics):
    _, s_, vh = svd(X)            # X: [N=65536, 16]
    orig = X @ vh
    out  = orig * std(orig, axis=0) + mean(orig, axis=0)   -> reshape [64,64,256]

Identities (same as the f32 baseline): out = X @ W + mean with W = vh * std,
mean/std computed in closed form from the SVD factors on host.  The SVD runs
on host via jax-CPU (LAPACK sgesdd sign conventions must match the reference).

Device formulation ("Y^T layout"): host pre-transposes X to XT [16, N],
converts to bf16, and packs it into per-core DRAM tiles so that each
[128, 512] sub-tile R[(m,k), f] = XT[k, n0 + 512m + f] covers 4096
consecutive rows n of one sample.  A single matmul with stationary
lhsT = kron(I8, W) gives
    P[(m,j), f] = sum_k W[k,j] X[n0+512m+f, k] = Y[n0+512m+f, j]
so the output tile DMAs back to DRAM contiguously in the same packed layout,
which host unpacks to Y [N, 16] f32.

This removes the PE transpose pass and the PSUM->SBUF copy of the f32
baseline, and bf16 in/out halves HBM traffic (the binding constraint):
per core 8 MiB in + 8 MiB out ~= 44 us at the ~390 GB/s/core effective DMA
rate, plus ~9 us fixed NEFF startup.

DMA plan (measured behavior drives this):
  - each DIRECT2D dma_start costs ~0.9 us on the issuing sequencer, and each
    DMA's completion adds ~0.3 us to ring 15 (the stripe's last ring), so
    DMAs are batched big: 8 input DMAs of 1 MiB, 7 output DMAs of 1 MiB.
  - the final 8 groups go out as 4 x 256 KiB DMAs so the drain after the
    last matmul/add is fine-grained.
  - only plain 2D [128, F] tiles (contiguous in DRAM) stripe evenly across
    all 16 rings; 3D APs were measured to use half the rings.
  - w/bias const DMAs are issued by sync BEFORE the input stream so PE's
    weights arrive with the first input tile, not 5 us later.

The bias add + f32->bf16 downcast (PSUM -> SBUF) alternates between the DVE
(tensor_scalar add) and the scalar/ACT engine (activation Identity with a
per-partition bias AP) so neither engine becomes the bottleneck.  The gpsimd
engine issues output DMAs.

Raw Bass (explicit per-engine programs + semaphores), as walrus only allows
one attached sync-wait per Matmult.
"""

import sys

import numpy as np

sys.path.insert(0, "/opt/trn_rl_repo")

B = 32
N = 65536
NC = 16
CORES = 8
SPC = B // CORES          # samples per core
GPS = 16                  # groups per sample ([128,512] bf16 tile = 4096 rows)
G = SPC * GPS             # 64 groups per core
FREE = 512
M = 8                     # 512-row blocks per group
BG = 8                    # groups per big DMA
H = G // BG               # 8 big DMAs per core per direction
WIDE = BG * FREE          # 4096
NTAIL = 4                 # tail out-DMAs (2 groups each) for the last big
TG = BG // NTAIL          # groups per tail DMA

IB = 4    # in-tile big slots (each [128, WIDE])
OTB = 4   # out-tile big slots
OB = 8    # matmul PSUM banks
LIN = 16
LOUT = 16

TRACE = False             # test.py sets this for profiling runs
LAST_EXEC_NS = None       # filled when TRACE

_compiled = None


def _build_graph():
    import concourse.bass as bass
    import concourse.mybir as mybir

    f32 = mybir.dt.float32
    bf16 = mybir.dt.bfloat16

    nc = bass.Bass()

    x_d = nc.declare_dram_parameter("x", [H, 128, WIDE], bf16, isOutput=False)
    w_d = nc.declare_dram_parameter("w", [SPC, 128, 128], bf16, isOutput=False)
    b_d = nc.declare_dram_parameter("bias", [128, SPC], f32, isOutput=False)
    om_d = nc.declare_dram_parameter("out_main", [H - 1, 128, WIDE], bf16, isOutput=True)
    ot_d = nc.declare_dram_parameter("out_tail", [NTAIL, 128, TG * FREE], bf16, isOutput=True)

    from contextlib import ExitStack

    with ExitStack() as ctx:
        w_sb = ctx.enter_context(nc.sbuf_tensor([128, SPC * 128], bf16))
        bias_sb = ctx.enter_context(nc.sbuf_tensor([128, SPC], f32))
        in_t = ctx.enter_context(nc.sbuf_tensor([128, IB * WIDE], bf16))
        ot_t = ctx.enter_context(nc.sbuf_tensor([128, OTB * WIDE], bf16))
        op = [ctx.enter_context(nc.psum_tensor(f"op{i}", [128, FREE], f32)) for i in range(OB)]
        s_const = ctx.enter_context(nc.semaphore())
        s_mm = ctx.enter_context(nc.semaphore())
        s_add_e = ctx.enter_context(nc.semaphore())
        s_add_o = ctx.enter_context(nc.semaphore())
        s_in = [ctx.enter_context(nc.semaphore(f"s_in{i}")) for i in range(LIN)]
        s_out = [ctx.enter_context(nc.semaphore(f"s_out{i}")) for i in range(LOUT)]
        block = ctx.enter_context(nc.Block())

        def in_sl(g):
            # group g's [128, FREE] slice within its big slot
            a = (g // BG % IB) * WIDE + (g % BG) * FREE
            return in_t[:, a : a + FREE]

        def in_big(h):
            return in_t[:, (h % IB) * WIDE : (h % IB + 1) * WIDE]

        def ot_sl(g):
            a = (g // BG % OTB) * WIDE + (g % BG) * FREE
            return ot_t[:, a : a + FREE]

        def ot_big(h):
            return ot_t[:, (h % OTB) * WIDE : (h % OTB + 1) * WIDE]

        def wait_add(eng, g_prev):
            eng.wait_ge(s_add_e if g_prev % 2 == 0 else s_add_o, g_prev // 2 + 1)

        @block.sync
        def _(sync):
            sync.dma_start(
                out=w_sb[:].rearrange("p (s f) -> p s f", s=SPC),
                in_=w_d[:].rearrange("s p f -> p s f"),
            ).then_inc(s_const, 16)
            sync.dma_start(out=bias_sb[:], in_=b_d[:]).then_inc(s_const, 16)
            for h in range(H):
                if h >= IB:
                    # last matmul consuming big slot h-IB
                    sync.wait_ge(s_mm, BG * (h - IB) + BG)
                sync.dma_start(out=in_big(h), in_=x_d[h]).then_inc(s_in[h % LIN], 16)

        @block.tensor
        def _(pe):
            pe.wait_ge(s_const, 32)
            for g in range(G):
                h = g // BG
                if g % BG == 0:
                    pe.wait_ge(s_in[h % LIN], 16 * (h // LIN + 1))
                if g >= OB:
                    wait_add(pe, g - OB)
                s = g // GPS
                nc.tensor.matmul(
                    op[g % OB][:],
                    lhsT=w_sb[:, s * 128 : (s + 1) * 128],
                    rhs=in_sl(g),
                    start=True,
                    stop=True,
                ).then_inc(s_mm, 1)

        @block.vector
        def _(dve):
            dve.wait_ge(s_const, 32)
            for g in range(0, G, 2):
                dve.wait_ge(s_mm, g + 1)
                if g >= OTB * BG:
                    hp = (g - OTB * BG) // BG  # big slot's previous user
                    dve.wait_ge(s_out[hp % LOUT], 16 * (hp // LOUT + 1))
                s = g // GPS
                nc.vector.tensor_scalar_add(
                    ot_sl(g), op[g % OB][:], bias_sb[:, s : s + 1]
                ).then_inc(s_add_e, 1)

        @block.scalar
        def _(act):
            act.wait_ge(s_const, 32)
            for g in range(1, G, 2):
                act.wait_ge(s_mm, g + 1)
                if g >= OTB * BG:
                    hp = (g - OTB * BG) // BG
                    act.wait_ge(s_out[hp % LOUT], 16 * (hp // LOUT + 1))
                s = g // GPS
                nc.scalar.activation(
                    ot_sl(g),
                    op[g % OB][:],
                    func=mybir.ActivationFunctionType.Identity,
                    bias=bias_sb[:, s : s + 1],
                    scale=1.0,
                ).then_inc(s_add_o, 1)

        @block.gpsimd
        def _(gp):
            for h in range(H - 1):
                # all adds of big h done: groups BG*h .. BG*h+BG-1
                gp.wait_ge(s_add_e, BG // 2 * (h + 1))
                gp.wait_ge(s_add_o, BG // 2 * (h + 1))
                gp.dma_start(out=om_d[h], in_=ot_big(h)).then_inc(s_out[h % LOUT], 16)
            h = H - 1
            base = ot_big(h)
            for t in range(NTAIL):
                g_hi = BG * h + TG * (t + 1) - 1  # last group in this tail chunk
                gp.wait_ge(s_add_e, g_hi // 2 + 1)
                gp.wait_ge(s_add_o, g_hi // 2 + 1)
                gp.dma_start(
                    out=ot_d[t], in_=base[:, t * TG * FREE : (t + 1) * TG * FREE]
                ).then_inc(s_out[(h + t) % LOUT], 16)

    return nc


def _to_bf16(a):
    """f32 contiguous -> bf16 (round-to-nearest-even), fast numpy path."""
    import ml_dtypes

    u = np.ascontiguousarray(a, np.float32).view(np.uint32)
    v = ((u + np.uint32(0x7FFF) + ((u >> np.uint32(16)) & np.uint32(1))) >> np.uint32(16)).astype(
        np.uint16
    )
    return v.view(ml_dtypes.bfloat16)


def _host_factors(x):
    """Per-sample affine factors: kron(I8, vh*std) [128,128] bf16, bias col [128] f32.

    The SVD must run through jax-CPU (jaxlib's LAPACK sgesdd) because the
    reference's output depends on the singular-vector sign conventions of that
    exact implementation.
    """
    import jax
    import jax.numpy as jnp

    cpu = jax.devices("cpu")[0]
    _, svs, vhs = jax.jit(
        lambda a: jnp.linalg.svd(a, full_matrices=False), device=cpu
    )(jax.device_put(x, cpu))
    svs = np.asarray(svs)
    vhs = np.asarray(vhs)

    import ml_dtypes

    ws = np.empty((B, 128, 128), ml_dtypes.bfloat16)
    bs = np.empty((B, 128), np.float32)
    eye8 = np.eye(8, dtype=np.float64)
    for s in range(B):
        Xs = x[s]
        sv, vh = svs[s], vhs[s]
        vh64 = vh.astype(np.float64)
        Mm = vh64 @ vh64
        xbar = Xs.mean(axis=0, dtype=np.float64)
        mean = xbar @ vh64
        e2 = (sv.astype(np.float64) ** 2) @ (Mm**2) / N
        var = np.maximum(e2 - mean**2, 0.0)
        std = np.sqrt(var)
        W = vh64 * std[None, :]
        ws[s] = np.kron(eye8, W).astype(ml_dtypes.bfloat16)
        bs[s] = np.tile(mean, 8).astype(np.float32)
    return ws, bs


def _pack(xtb_core):
    """[SPC, 16, N] bf16 -> [H, 128, WIDE]: x[h, 16m+k, 512i+f] = XT[s, k, n+...].

    h = (H//SPC)*s + hl, sub-group i in 0..BG-1, n = (BG*hl + i)*4096 + 512m + f.
    """
    v = xtb_core.view(np.uint16)
    # (s, k, hl, i, m, f)
    v6 = v.reshape(SPC, NC, H // SPC, BG, M, FREE)
    # -> (s, hl, m, k, i, f)
    t = v6.transpose(0, 2, 4, 1, 3, 5)
    return np.ascontiguousarray(t).reshape(H, 128, WIDE).view(xtb_core.dtype)


def _unpack(o_all_u16):
    """[H, 128, WIDE] uint16 (device layout, j in place of k) -> [SPC, 16, N]."""
    v6 = o_all_u16.reshape(SPC, H // SPC, M, NC, BG, FREE)  # (s, hl, m, j, i, f)
    t = v6.transpose(0, 3, 1, 4, 2, 5)                      # (s, j, hl, i, m, f)
    return np.ascontiguousarray(t).reshape(SPC, NC, N)


def kernel(x):
    global _compiled, LAST_EXEC_NS
    from concourse.bass_utils import run_bass_kernel_spmd

    x = np.ascontiguousarray(np.asarray(x), dtype=np.float32).reshape(B, N, NC)
    ws, bs = _host_factors(x)

    xt = np.ascontiguousarray(x.transpose(0, 2, 1))  # [B, 16, N] f32
    xtb = _to_bf16(xt).reshape(B, NC, N)             # [B, 16, N] bf16

    if _compiled is None:
        _compiled = _build_graph()
    nc = _compiled

    in_maps = []
    for c in range(CORES):
        s0 = c * SPC
        in_maps.append(
            {
                "x": _pack(xtb[s0 : s0 + SPC]),
                "w": ws[s0 : s0 + SPC],
                "bias": np.ascontiguousarray(bs[s0 : s0 + SPC].T),
            }
        )

    res = run_bass_kernel_spmd(nc, in_maps, core_ids=list(range(CORES)), trace=TRACE)
    LAST_EXEC_NS = res.exec_time_ns

    yt_u = np.empty((B, NC, N), np.uint16)
    for c in range(CORES):
        o_all = np.empty((H, 128, WIDE), np.uint16)
        o_all[: H - 1] = np.asarray(res.results[c]["out_main"]).view(np.uint16)
        tail = np.asarray(res.results[c]["out_tail"]).view(np.uint16)
        o_all[H - 1] = tail.transpose(1, 0, 2).reshape(128, WIDE)
        yt_u[c * SPC : (c + 1) * SPC] = _unpack(o_all)
    yf = (yt_u.astype(np.uint32) << np.uint32(16)).view(np.float32)  # [B,16,N] f32
    out = np.ascontiguousarray(yf.transpose(0, 2, 1))                # [B,N,16]
    return out.reshape(B, 64, 64, 256)


# revision 7
# speedup vs baseline: 1.1023x; 1.1023x over previous
"""BPCA Unpooling kernel for Trainium2 (8 NeuronCores, data-parallel over batch).

Math per sample s (reference semantics):
    _, s_, vh = svd(X)            # X: [N=65536, 16]
    orig = X @ vh
    out  = orig * std(orig, axis=0) + mean(orig, axis=0)   -> reshape [64,64,256]

Identities (same as the f32 baseline): out = X @ W + mean with W = vh * std,
mean/std computed in closed form from the SVD factors on host.  The SVD runs
on host via jax-CPU (LAPACK sgesdd sign conventions must match the reference).

Device formulation ("Y^T layout"): host pre-transposes X to XT [16, N],
converts to bf16, and packs it into per-core DRAM tiles so that each
[128, 512] sub-tile ("group", 4096 rows) R[(m,k), f] = XT[k, n0 + 512m + f].
A single matmul per group with stationary lhsT = kron(I8, W) gives
    P[(m,j), f] = sum_k W[k,j] X[n0+512m+f, k] = Y[n0+512m+f, j]
so output tiles DMA back to DRAM contiguously in the same packed layout,
which host unpacks to Y [N, 16] f32.

This removes the PE transpose pass and the PSUM->SBUF copy of the f32
baseline, and bf16 in/out halves HBM traffic (the binding constraint):
per core 8 MiB in + 8 MiB out ~= 43 us at the ~390 GB/s/core effective DMA
rate, plus ~9 us fixed NEFF startup.

DMA plan (measured):
  - each DIRECT2D dma_start costs ~0.9 us on the issuing sequencer and each
    DMA completion adds ~0.3 us to ring 15 (last ring of the stripe), so the
    steady-state stream uses big 1 MiB (8-group) DMAs;
  - head and tail use small 2-group (256 KiB) DMAs so the first matmul isn't
    gated on a fat chunk and the drain after the last add is fine-grained;
  - only plain 2D [128, F] tiles (contiguous in DRAM) stripe evenly across
    all 16 rings (3D APs were measured to use half the rings), hence one
    DRAM parameter per chunk-size class;
  - w/bias const DMAs are issued by sync BEFORE the input stream so PE's
    weights arrive with the first input tile.

The bias add + f32->bf16 downcast (PSUM -> SBUF) alternates between the DVE
(tensor_scalar add) and the scalar/ACT engine (activation Identity with a
per-partition bias AP).  The gpsimd engine issues output DMAs.

Raw Bass (explicit per-engine programs + semaphores), as walrus only allows
one attached sync-wait per Matmult.
"""

import sys

import numpy as np

sys.path.insert(0, "/opt/trn_rl_repo")

B = 32
N = 65536
NC = 16
CORES = 8
SPC = B // CORES          # samples per core
GPS = 16                  # groups per sample
G = SPC * GPS             # 64 groups per core
FREE = 512
M = 8                     # 512-row blocks per group

# chunk schedules, in groups (each group = 128 KiB bf16)
IN_CHUNKS = [2, 2, 4, 8, 8, 8, 8, 8, 8, 4, 2, 2]
OUT_CHUNKS = [8, 8, 8, 8, 8, 8, 8, 4, 2, 2]
assert sum(IN_CHUNKS) == G and sum(OUT_CHUNKS) == G

IBG = 24   # in-tile ring, group slots
OTG = 24   # out-tile ring, group slots
OB = 6     # matmul PSUM banks
LIN = 16
LOUT = 16


def _starts(chunks):
    s, out = 0, []
    for c in chunks:
        out.append(s)
        s += c
    return out


IN_STARTS = _starts(IN_CHUNKS)
OUT_STARTS = _starts(OUT_CHUNKS)
for _s, _c in zip(IN_STARTS, IN_CHUNKS):
    assert _s % IBG + _c <= IBG and _s % GPS + _c <= GPS
for _s, _c in zip(OUT_STARTS, OUT_CHUNKS):
    assert _s % OTG + _c <= OTG and _s % GPS + _c <= GPS


def _classes(chunks):
    """chunk list -> {size: count}, and per-chunk (size, index-within-size)."""
    counts, refs = {}, []
    for c in chunks:
        i = counts.get(c, 0)
        refs.append((c, i))
        counts[c] = i + 1
    return counts, refs


IN_COUNTS, IN_REFS = _classes(IN_CHUNKS)
OUT_COUNTS, OUT_REFS = _classes(OUT_CHUNKS)

TRACE = False             # test.py sets this for profiling runs
LAST_EXEC_NS = None       # filled when TRACE

_compiled = None


def _build_graph():
    import concourse.bass as bass
    import concourse.mybir as mybir

    f32 = mybir.dt.float32
    bf16 = mybir.dt.bfloat16

    nc = bass.Bass()

    w_d = nc.declare_dram_parameter("w", [SPC, 128, 128], bf16, isOutput=False)
    b_d = nc.declare_dram_parameter("bias", [128, SPC], f32, isOutput=False)
    x_cls = {
        c: nc.declare_dram_parameter(f"x{c}", [n, 128, c * FREE], bf16, isOutput=False)
        for c, n in IN_COUNTS.items()
    }
    o_cls = {
        c: nc.declare_dram_parameter(f"o{c}", [n, 128, c * FREE], bf16, isOutput=True)
        for c, n in OUT_COUNTS.items()
    }

    from contextlib import ExitStack

    with ExitStack() as ctx:
        w_sb = ctx.enter_context(nc.sbuf_tensor([128, SPC * 128], bf16))
        bias_sb = ctx.enter_context(nc.sbuf_tensor([128, SPC], f32))
        in_t = ctx.enter_context(nc.sbuf_tensor([128, IBG * FREE], bf16))
        ot_t = ctx.enter_context(nc.sbuf_tensor([128, OTG * FREE], bf16))
        op = [ctx.enter_context(nc.psum_tensor(f"op{i}", [128, FREE], f32)) for i in range(OB)]
        s_const = ctx.enter_context(nc.semaphore())
        s_mm = ctx.enter_context(nc.semaphore())
        s_add_e = ctx.enter_context(nc.semaphore())
        s_add_o = ctx.enter_context(nc.semaphore())
        s_in = [ctx.enter_context(nc.semaphore(f"s_in{i}")) for i in range(LIN)]
        s_out = [ctx.enter_context(nc.semaphore(f"s_out{i}")) for i in range(LOUT)]
        block = ctx.enter_context(nc.Block())

        def in_sl(g):
            a = (g % IBG) * FREE
            return in_t[:, a : a + FREE]

        def ot_sl(g):
            a = (g % OTG) * FREE
            return ot_t[:, a : a + FREE]

        # out-chunk index containing group g
        def out_chunk_of(g):
            for j, (s, c) in enumerate(zip(OUT_STARTS, OUT_CHUNKS)):
                if s <= g < s + c:
                    return j
            raise AssertionError(g)

        def wait_add(eng, g_prev):
            eng.wait_ge(s_add_e if g_prev % 2 == 0 else s_add_o, g_prev // 2 + 1)

        @block.sync
        def _(sync):
            sync.dma_start(
                out=w_sb[:].rearrange("p (s f) -> p s f", s=SPC),
                in_=w_d[:].rearrange("s p f -> p s f"),
            ).then_inc(s_const, 16)
            sync.dma_start(out=bias_sb[:], in_=b_d[:]).then_inc(s_const, 16)
            for j, (gs, c) in enumerate(zip(IN_STARTS, IN_CHUNKS)):
                if gs + c > IBG:
                    sync.wait_ge(s_mm, gs + c - IBG)
                (sz, idx) = IN_REFS[j]
                a = (gs % IBG) * FREE
                sync.dma_start(
                    out=in_t[:, a : a + c * FREE], in_=x_cls[sz][idx]
                ).then_inc(s_in[j % LIN], 16)

        @block.tensor
        def _(pe):
            pe.wait_ge(s_const, 32)
            for g in range(G):
                if g in IN_STARTS:
                    j = IN_STARTS.index(g)
                    pe.wait_ge(s_in[j % LIN], 16 * (j // LIN + 1))
                if g >= OB:
                    wait_add(pe, g - OB)
                s = g // GPS
                nc.tensor.matmul(
                    op[g % OB][:],
                    lhsT=w_sb[:, s * 128 : (s + 1) * 128],
                    rhs=in_sl(g),
                    start=True,
                    stop=True,
                ).then_inc(s_mm, 1)

        def add_body(eng, g, emit):
            eng.wait_ge(s_mm, g + 1)
            if g >= OTG:
                dj = out_chunk_of(g - OTG)
                eng.wait_ge(s_out[dj % LOUT], 16 * (dj // LOUT + 1))
            emit(g // GPS)

        @block.vector
        def _(dve):
            dve.wait_ge(s_const, 32)
            for g in range(0, G, 2):
                add_body(
                    dve,
                    g,
                    lambda s, g=g: nc.vector.tensor_scalar_add(
                        ot_sl(g), op[g % OB][:], bias_sb[:, s : s + 1]
                    ).then_inc(s_add_e, 1),
                )

        @block.scalar
        def _(act):
            import concourse.mybir as mybir

            act.wait_ge(s_const, 32)
            for g in range(1, G, 2):
                add_body(
                    act,
                    g,
                    lambda s, g=g: nc.scalar.activation(
                        ot_sl(g),
                        op[g % OB][:],
                        func=mybir.ActivationFunctionType.Identity,
                        bias=bias_sb[:, s : s + 1],
                        scale=1.0,
                    ).then_inc(s_add_o, 1),
                )

        @block.gpsimd
        def _(gp):
            for j, (gs, c) in enumerate(zip(OUT_STARTS, OUT_CHUNKS)):
                ge = gs + c  # one past last group of chunk
                gp.wait_ge(s_add_e, (ge + 1) // 2)
                gp.wait_ge(s_add_o, ge // 2)
                (sz, idx) = OUT_REFS[j]
                a = (gs % OTG) * FREE
                gp.dma_start(
                    out=o_cls[sz][idx], in_=ot_t[:, a : a + c * FREE]
                ).then_inc(s_out[j % LOUT], 16)

    return nc


def _to_bf16(a):
    """f32 contiguous -> bf16 (round-to-nearest-even), fast numpy path."""
    import ml_dtypes

    u = np.ascontiguousarray(a, np.float32).view(np.uint32)
    v = ((u + np.uint32(0x7FFF) + ((u >> np.uint32(16)) & np.uint32(1))) >> np.uint32(16)).astype(
        np.uint16
    )
    return v.view(ml_dtypes.bfloat16)


def _host_factors(x):
    """Per-sample affine factors: kron(I8, vh*std) [128,128] bf16, bias col [128] f32.

    The SVD must run through jax-CPU (jaxlib's LAPACK sgesdd) because the
    reference's output depends on the singular-vector sign conventions of that
    exact implementation.
    """
    import jax
    import jax.numpy as jnp

    cpu = jax.devices("cpu")[0]
    _, svs, vhs = jax.jit(
        lambda a: jnp.linalg.svd(a, full_matrices=False), device=cpu
    )(jax.device_put(x, cpu))
    svs = np.asarray(svs)
    vhs = np.asarray(vhs)

    import ml_dtypes

    ws = np.empty((B, 128, 128), ml_dtypes.bfloat16)
    bs = np.empty((B, 128), np.float32)
    eye8 = np.eye(8, dtype=np.float64)
    for s in range(B):
        Xs = x[s]
        sv, vh = svs[s], vhs[s]
        vh64 = vh.astype(np.float64)
        Mm = vh64 @ vh64
        xbar = Xs.mean(axis=0, dtype=np.float64)
        mean = xbar @ vh64
        e2 = (sv.astype(np.float64) ** 2) @ (Mm**2) / N
        var = np.maximum(e2 - mean**2, 0.0)
        std = np.sqrt(var)
        W = vh64 * std[None, :]
        ws[s] = np.kron(eye8, W).astype(ml_dtypes.bfloat16)
        bs[s] = np.tile(mean, 8).astype(np.float32)
    return ws, bs


def _pack_core(xtb_core):
    """[SPC, 16, N] bf16 -> {size: [n, 128, size*FREE] uint16} per IN_CHUNKS."""
    v = xtb_core.view(np.uint16)
    arrs = {c: np.empty((n, 128, c * FREE), np.uint16) for c, n in IN_COUNTS.items()}
    for j, (gs, c) in enumerate(zip(IN_STARTS, IN_CHUNKS)):
        s, n0 = gs // GPS, (gs % GPS) * (M * FREE)
        seg = v[s][:, n0 : n0 + c * M * FREE]          # [16, c*4096]
        t = seg.reshape(NC, c, M, FREE).transpose(2, 0, 1, 3)  # (m,k,i,f)
        sz, idx = IN_REFS[j]
        arrs[sz][idx] = t.reshape(128, c * FREE)
    return arrs


def _unpack_core(res_core):
    """device outputs -> [SPC, 16, N] uint16 (YT layout)."""
    yt = np.empty((SPC, NC, N), np.uint16)
    for j, (gs, c) in enumerate(zip(OUT_STARTS, OUT_CHUNKS)):
        sz, idx = OUT_REFS[j]
        tile = np.asarray(res_core[f"o{sz}"][idx]).view(np.uint16)  # [128, c*FREE]
        s, n0 = gs // GPS, (gs % GPS) * (M * FREE)
        seg = tile.reshape(M, NC, c, FREE).transpose(1, 2, 0, 3)    # (j,i,m,f)
        yt[s][:, n0 : n0 + c * M * FREE] = seg.reshape(NC, c * M * FREE)
    return yt


def kernel(x):
    global _compiled, LAST_EXEC_NS
    from concourse.bass_utils import run_bass_kernel_spmd

    x = np.ascontiguousarray(np.asarray(x), dtype=np.float32).reshape(B, N, NC)
    ws, bs = _host_factors(x)

    xt = np.ascontiguousarray(x.transpose(0, 2, 1))  # [B, 16, N] f32
    xtb = _to_bf16(xt).reshape(B, NC, N)             # [B, 16, N] bf16

    if _compiled is None:
        _compiled = _build_graph()
    nc = _compiled

    import ml_dtypes

    in_maps = []
    for c in range(CORES):
        s0 = c * SPC
        m = {
            "w": ws[s0 : s0 + SPC],
            "bias": np.ascontiguousarray(bs[s0 : s0 + SPC].T),
        }
        for sz, arr in _pack_core(xtb[s0 : s0 + SPC]).items():
            m[f"x{sz}"] = arr.view(ml_dtypes.bfloat16)
        in_maps.append(m)

    res = run_bass_kernel_spmd(nc, in_maps, core_ids=list(range(CORES)), trace=TRACE)
    LAST_EXEC_NS = res.exec_time_ns

    yt_u = np.empty((B, NC, N), np.uint16)
    for c in range(CORES):
        yt_u[c * SPC : (c + 1) * SPC] = _unpack_core(res.results[c])
    yf = (yt_u.astype(np.uint32) << np.uint32(16)).view(np.float32)  # [B,16,N] f32
    out = np.ascontiguousarray(yf.transpose(0, 2, 1))                # [B,N,16]
    return out.reshape(B, 64, 64, 256)


# revision 8
# speedup vs baseline: 1.1461x; 1.0397x over previous
"""BPCA Unpooling kernel for Trainium2 (8 NeuronCores, data-parallel over batch).

Math per sample s (reference semantics):
    _, s_, vh = svd(X)            # X: [N=65536, 16]
    orig = X @ vh
    out  = orig * std(orig, axis=0) + mean(orig, axis=0)   -> reshape [64,64,256]

Identities (same as the f32 baseline): out = X @ W + mean with W = vh * std,
mean/std computed in closed form from the SVD factors on host.  The SVD runs
on host via jax-CPU (LAPACK sgesdd sign conventions must match the reference).

Device formulation ("Y^T layout"): host pre-transposes X to XT [16, N],
converts to bf16, and packs it into per-core DRAM tiles so that each
[128, 512] sub-tile ("group", 4096 rows) R[(m,k), f] = XT[k, n0 + 512m + f].
A single matmul per group with stationary lhsT = kron(I8, W) gives
    P[(m,j), f] = sum_k W[k,j] X[n0+512m+f, k] = Y[n0+512m+f, j]
so output tiles DMA back to DRAM contiguously in the same packed layout,
which host unpacks to Y [N, 16] f32.

This removes the PE transpose pass and the PSUM->SBUF copy of the f32
baseline, and bf16 in/out halves HBM traffic (the binding constraint):
per core 8 MiB in + 8 MiB out ~= 43 us at the ~390 GB/s/core effective DMA
rate, plus ~9 us fixed NEFF startup.

DMA plan (measured):
  - each DIRECT2D dma_start costs ~0.9 us on the issuing sequencer and each
    DMA completion adds ~0.3 us to ring 15 (last ring of the stripe), so the
    steady-state stream uses big 1 MiB (8-group) DMAs;
  - head and tail use small 2-group (256 KiB) DMAs so the first matmul isn't
    gated on a fat chunk and the drain after the last add is fine-grained;
  - only plain 2D [128, F] tiles (contiguous in DRAM) stripe evenly across
    all 16 rings (3D APs were measured to use half the rings), hence one
    DRAM parameter per chunk-size class;
  - w/bias const DMAs are issued by sync BEFORE the input stream so PE's
    weights arrive with the first input tile.

The bias add + f32->bf16 downcast (PSUM -> SBUF) alternates between the DVE
(tensor_scalar add) and the scalar/ACT engine (activation Identity with a
per-partition bias AP).  The gpsimd engine issues output DMAs.

Raw Bass (explicit per-engine programs + semaphores), as walrus only allows
one attached sync-wait per Matmult.
"""

import sys

import numpy as np

sys.path.insert(0, "/opt/trn_rl_repo")

B = 32
N = 65536
NC = 16
CORES = 8
SPC = B // CORES          # samples per core
GPS = 16                  # groups per sample
G = SPC * GPS             # 64 groups per core
FREE = 512
M = 8                     # 512-row blocks per group

# chunk schedules, in groups (each group = 128 KiB bf16)
IN_CHUNKS = [4, 4, 8, 8, 8, 8, 8, 8, 4, 2, 2]
OUT_CHUNKS = [8, 8, 8, 8, 8, 8, 8, 4, 2, 2]
assert sum(IN_CHUNKS) == G and sum(OUT_CHUNKS) == G

IBG = 48   # in-tile ring, group slots
OTG = 48   # out-tile ring, group slots
OB = 6     # matmul PSUM banks
LIN = 16
LOUT = 16


def _starts(chunks):
    s, out = 0, []
    for c in chunks:
        out.append(s)
        s += c
    return out


IN_STARTS = _starts(IN_CHUNKS)
OUT_STARTS = _starts(OUT_CHUNKS)
for _s, _c in zip(IN_STARTS, IN_CHUNKS):
    assert _s % IBG + _c <= IBG and _s % GPS + _c <= GPS
for _s, _c in zip(OUT_STARTS, OUT_CHUNKS):
    assert _s % OTG + _c <= OTG and _s % GPS + _c <= GPS


def _classes(chunks):
    """chunk list -> {size: count}, and per-chunk (size, index-within-size)."""
    counts, refs = {}, []
    for c in chunks:
        i = counts.get(c, 0)
        refs.append((c, i))
        counts[c] = i + 1
    return counts, refs


IN_COUNTS, IN_REFS = _classes(IN_CHUNKS)
OUT_COUNTS, OUT_REFS = _classes(OUT_CHUNKS)

TRACE = False             # test.py sets this for profiling runs
LAST_EXEC_NS = None       # filled when TRACE

_compiled = None


def _build_graph():
    import concourse.bass as bass
    import concourse.mybir as mybir

    f32 = mybir.dt.float32
    bf16 = mybir.dt.bfloat16

    nc = bass.Bass()

    w_d = nc.declare_dram_parameter("w", [SPC, 128, 128], bf16, isOutput=False)
    b_d = nc.declare_dram_parameter("bias", [128, SPC], f32, isOutput=False)
    x_cls = {
        c: nc.declare_dram_parameter(f"x{c}", [n, 128, c * FREE], bf16, isOutput=False)
        for c, n in IN_COUNTS.items()
    }
    o_cls = {
        c: nc.declare_dram_parameter(f"o{c}", [n, 128, c * FREE], bf16, isOutput=True)
        for c, n in OUT_COUNTS.items()
    }

    from contextlib import ExitStack

    with ExitStack() as ctx:
        w_sb = ctx.enter_context(nc.sbuf_tensor([128, SPC * 128], bf16))
        bias_sb = ctx.enter_context(nc.sbuf_tensor([128, SPC], f32))
        in_t = ctx.enter_context(nc.sbuf_tensor([128, IBG * FREE], bf16))
        ot_t = ctx.enter_context(nc.sbuf_tensor([128, OTG * FREE], bf16))
        op = [ctx.enter_context(nc.psum_tensor(f"op{i}", [128, FREE], f32)) for i in range(OB)]
        s_const = ctx.enter_context(nc.semaphore())
        s_mm = ctx.enter_context(nc.semaphore())
        s_add_e = ctx.enter_context(nc.semaphore())
        s_add_o = ctx.enter_context(nc.semaphore())
        s_in = [ctx.enter_context(nc.semaphore(f"s_in{i}")) for i in range(LIN)]
        s_out = [ctx.enter_context(nc.semaphore(f"s_out{i}")) for i in range(LOUT)]
        block = ctx.enter_context(nc.Block())

        def in_sl(g):
            a = (g % IBG) * FREE
            return in_t[:, a : a + FREE]

        def ot_sl(g):
            a = (g % OTG) * FREE
            return ot_t[:, a : a + FREE]

        # out-chunk index containing group g
        def out_chunk_of(g):
            for j, (s, c) in enumerate(zip(OUT_STARTS, OUT_CHUNKS)):
                if s <= g < s + c:
                    return j
            raise AssertionError(g)

        def wait_add(eng, g_prev):
            eng.wait_ge(s_add_e if g_prev % 2 == 0 else s_add_o, g_prev // 2 + 1)

        @block.sync
        def _(sync):
            sync.dma_start(
                out=w_sb[:].rearrange("p (s f) -> p s f", s=SPC),
                in_=w_d[:].rearrange("s p f -> p s f"),
            ).then_inc(s_const, 16)
            sync.dma_start(out=bias_sb[:], in_=b_d[:]).then_inc(s_const, 16)
            for j, (gs, c) in enumerate(zip(IN_STARTS, IN_CHUNKS)):
                if gs + c > IBG:
                    sync.wait_ge(s_mm, gs + c - IBG)
                (sz, idx) = IN_REFS[j]
                a = (gs % IBG) * FREE
                sync.dma_start(
                    out=in_t[:, a : a + c * FREE], in_=x_cls[sz][idx]
                ).then_inc(s_in[j % LIN], 16)

        @block.tensor
        def _(pe):
            pe.wait_ge(s_const, 32)
            for g in range(G):
                if g in IN_STARTS:
                    j = IN_STARTS.index(g)
                    pe.wait_ge(s_in[j % LIN], 16 * (j // LIN + 1))
                if g >= OB:
                    wait_add(pe, g - OB)
                s = g // GPS
                nc.tensor.matmul(
                    op[g % OB][:],
                    lhsT=w_sb[:, s * 128 : (s + 1) * 128],
                    rhs=in_sl(g),
                    start=True,
                    stop=True,
                ).then_inc(s_mm, 1)

        def add_body(eng, g, emit):
            eng.wait_ge(s_mm, g + 1)
            if g >= OTG:
                dj = out_chunk_of(g - OTG)
                eng.wait_ge(s_out[dj % LOUT], 16 * (dj // LOUT + 1))
            emit(g // GPS)

        @block.vector
        def _(dve):
            dve.wait_ge(s_const, 32)
            for g in range(0, G, 2):
                add_body(
                    dve,
                    g,
                    lambda s, g=g: nc.vector.tensor_scalar_add(
                        ot_sl(g), op[g % OB][:], bias_sb[:, s : s + 1]
                    ).then_inc(s_add_e, 1),
                )

        @block.scalar
        def _(act):
            import concourse.mybir as mybir

            act.wait_ge(s_const, 32)
            for g in range(1, G, 2):
                add_body(
                    act,
                    g,
                    lambda s, g=g: nc.scalar.activation(
                        ot_sl(g),
                        op[g % OB][:],
                        func=mybir.ActivationFunctionType.Identity,
                        bias=bias_sb[:, s : s + 1],
                        scale=1.0,
                    ).then_inc(s_add_o, 1),
                )

        @block.gpsimd
        def _(gp):
            for j, (gs, c) in enumerate(zip(OUT_STARTS, OUT_CHUNKS)):
                ge = gs + c  # one past last group of chunk
                gp.wait_ge(s_add_e, (ge + 1) // 2)
                gp.wait_ge(s_add_o, ge // 2)
                (sz, idx) = OUT_REFS[j]
                a = (gs % OTG) * FREE
                gp.dma_start(
                    out=o_cls[sz][idx], in_=ot_t[:, a : a + c * FREE]
                ).then_inc(s_out[j % LOUT], 16)

    return nc


def _to_bf16(a):
    """f32 contiguous -> bf16 (round-to-nearest-even), fast numpy path."""
    import ml_dtypes

    u = np.ascontiguousarray(a, np.float32).view(np.uint32)
    v = ((u + np.uint32(0x7FFF) + ((u >> np.uint32(16)) & np.uint32(1))) >> np.uint32(16)).astype(
        np.uint16
    )
    return v.view(ml_dtypes.bfloat16)


def _host_factors(x):
    """Per-sample affine factors: kron(I8, vh*std) [128,128] bf16, bias col [128] f32.

    The SVD must run through jax-CPU (jaxlib's LAPACK sgesdd) because the
    reference's output depends on the singular-vector sign conventions of that
    exact implementation.
    """
    import jax
    import jax.numpy as jnp

    cpu = jax.devices("cpu")[0]
    _, svs, vhs = jax.jit(
        lambda a: jnp.linalg.svd(a, full_matrices=False), device=cpu
    )(jax.device_put(x, cpu))
    svs = np.asarray(svs)
    vhs = np.asarray(vhs)

    import ml_dtypes

    ws = np.empty((B, 128, 128), ml_dtypes.bfloat16)
    bs = np.empty((B, 128), np.float32)
    eye8 = np.eye(8, dtype=np.float64)
    for s in range(B):
        Xs = x[s]
        sv, vh = svs[s], vhs[s]
        vh64 = vh.astype(np.float64)
        Mm = vh64 @ vh64
        xbar = Xs.mean(axis=0, dtype=np.float64)
        mean = xbar @ vh64
        e2 = (sv.astype(np.float64) ** 2) @ (Mm**2) / N
        var = np.maximum(e2 - mean**2, 0.0)
        std = np.sqrt(var)
        W = vh64 * std[None, :]
        ws[s] = np.kron(eye8, W).astype(ml_dtypes.bfloat16)
        bs[s] = np.tile(mean, 8).astype(np.float32)
    return ws, bs


def _pack_core(xtb_core):
    """[SPC, 16, N] bf16 -> {size: [n, 128, size*FREE] uint16} per IN_CHUNKS."""
    v = xtb_core.view(np.uint16)
    arrs = {c: np.empty((n, 128, c * FREE), np.uint16) for c, n in IN_COUNTS.items()}
    for j, (gs, c) in enumerate(zip(IN_STARTS, IN_CHUNKS)):
        s, n0 = gs // GPS, (gs % GPS) * (M * FREE)
        seg = v[s][:, n0 : n0 + c * M * FREE]          # [16, c*4096]
        t = seg.reshape(NC, c, M, FREE).transpose(2, 0, 1, 3)  # (m,k,i,f)
        sz, idx = IN_REFS[j]
        arrs[sz][idx] = t.reshape(128, c * FREE)
    return arrs


def _unpack_core(res_core):
    """device outputs -> [SPC, 16, N] uint16 (YT layout)."""
    yt = np.empty((SPC, NC, N), np.uint16)
    for j, (gs, c) in enumerate(zip(OUT_STARTS, OUT_CHUNKS)):
        sz, idx = OUT_REFS[j]
        tile = np.asarray(res_core[f"o{sz}"][idx]).view(np.uint16)  # [128, c*FREE]
        s, n0 = gs // GPS, (gs % GPS) * (M * FREE)
        seg = tile.reshape(M, NC, c, FREE).transpose(1, 2, 0, 3)    # (j,i,m,f)
        yt[s][:, n0 : n0 + c * M * FREE] = seg.reshape(NC, c * M * FREE)
    return yt


def kernel(x):
    global _compiled, LAST_EXEC_NS
    from concourse.bass_utils import run_bass_kernel_spmd

    x = np.ascontiguousarray(np.asarray(x), dtype=np.float32).reshape(B, N, NC)
    ws, bs = _host_factors(x)

    xt = np.ascontiguousarray(x.transpose(0, 2, 1))  # [B, 16, N] f32
    xtb = _to_bf16(xt).reshape(B, NC, N)             # [B, 16, N] bf16

    if _compiled is None:
        _compiled = _build_graph()
    nc = _compiled

    import ml_dtypes

    in_maps = []
    for c in range(CORES):
        s0 = c * SPC
        m = {
            "w": ws[s0 : s0 + SPC],
            "bias": np.ascontiguousarray(bs[s0 : s0 + SPC].T),
        }
        for sz, arr in _pack_core(xtb[s0 : s0 + SPC]).items():
            m[f"x{sz}"] = arr.view(ml_dtypes.bfloat16)
        in_maps.append(m)

    res = run_bass_kernel_spmd(nc, in_maps, core_ids=list(range(CORES)), trace=TRACE)
    LAST_EXEC_NS = res.exec_time_ns

    yt_u = np.empty((B, NC, N), np.uint16)
    for c in range(CORES):
        yt_u[c * SPC : (c + 1) * SPC] = _unpack_core(res.results[c])
    yf = (yt_u.astype(np.uint32) << np.uint32(16)).view(np.float32)  # [B,16,N] f32
    out = np.ascontiguousarray(yf.transpose(0, 2, 1))                # [B,N,16]
    return out.reshape(B, 64, 64, 256)


# revision 12
# speedup vs baseline: 1.1627x; 1.0145x over previous
"""BPCA Unpooling kernel for Trainium2 (8 NeuronCores, data-parallel over batch).

Math per sample s (reference semantics):
    _, s_, vh = svd(X)            # X: [N=65536, 16]
    orig = X @ vh
    out  = orig * std(orig, axis=0) + mean(orig, axis=0)   -> reshape [64,64,256]

Identities (same as the f32 baseline): out = X @ W + mean with W = vh * std,
mean/std computed in closed form from the SVD factors on host.  The SVD runs
on host via jax-CPU (LAPACK sgesdd sign conventions must match the reference).

Device formulation ("Y^T layout"): host pre-transposes X to XT [16, N],
converts to bf16, and packs it into per-core DRAM tiles so that each
[128, 512] sub-tile ("group", 4096 rows) R[(m,k), f] = XT[k, n0 + 512m + f].
A single matmul per group with stationary lhsT = kron(I8, W) gives
    P[(m,j), f] = sum_k W[k,j] X[n0+512m+f, k] = Y[n0+512m+f, j]
so output tiles DMA back to DRAM contiguously in the same packed layout,
which host unpacks to Y [N, 16] f32.

This removes the PE transpose pass and the PSUM->SBUF copy of the f32
baseline, and bf16 in/out halves HBM traffic (the binding constraint):
per core 8 MiB in + 8 MiB out ~= 43 us at the ~390 GB/s/core effective DMA
rate, plus ~9 us fixed NEFF startup.

DMA plan (measured):
  - each DIRECT2D dma_start costs ~0.9 us on the issuing sequencer and each
    DMA completion adds ~0.3 us to ring 15 (last ring of the stripe), so the
    steady-state stream uses big 1 MiB (8-group) DMAs;
  - head and tail use small 2-group (256 KiB) DMAs so the first matmul isn't
    gated on a fat chunk and the drain after the last add is fine-grained;
  - only plain 2D [128, F] tiles (contiguous in DRAM) stripe evenly across
    all 16 rings (3D APs were measured to use half the rings), hence one
    DRAM parameter per chunk-size class;
  - w/bias const DMAs are issued by sync BEFORE the input stream so PE's
    weights arrive with the first input tile.

The bias add + f32->bf16 downcast (PSUM -> SBUF) alternates between the DVE
(tensor_scalar add) and the scalar/ACT engine (activation Identity with a
per-partition bias AP).  The gpsimd engine issues output DMAs.

Raw Bass (explicit per-engine programs + semaphores), as walrus only allows
one attached sync-wait per Matmult.
"""

import sys

import numpy as np

sys.path.insert(0, "/opt/trn_rl_repo")

B = 32
N = 65536
NC = 16
CORES = 8
SPC = B // CORES          # samples per core
GPS = 16                  # groups per sample
G = SPC * GPS             # 64 groups per core
FREE = 512
M = 8                     # 512-row blocks per group

# chunk schedules, in groups (each group = 128 KiB bf16)
IN_CHUNKS = [4, 4, 8, 8, 8, 8, 8, 8, 4, 2, 1, 1]
OUT_CHUNKS = [8, 8, 8, 8, 8, 8, 8, 4, 2, 1, 1]
assert sum(IN_CHUNKS) == G and sum(OUT_CHUNKS) == G

IBG = 48   # in-tile ring, group slots
OTG = 48   # out-tile ring, group slots
OB = 6     # matmul PSUM banks
LIN = 16
LOUT = 16


def _starts(chunks):
    s, out = 0, []
    for c in chunks:
        out.append(s)
        s += c
    return out


IN_STARTS = _starts(IN_CHUNKS)
OUT_STARTS = _starts(OUT_CHUNKS)
for _s, _c in zip(IN_STARTS, IN_CHUNKS):
    assert _s % IBG + _c <= IBG and _s % GPS + _c <= GPS
for _s, _c in zip(OUT_STARTS, OUT_CHUNKS):
    assert _s % OTG + _c <= OTG and _s % GPS + _c <= GPS


def _classes(chunks):
    """chunk list -> {size: count}, and per-chunk (size, index-within-size)."""
    counts, refs = {}, []
    for c in chunks:
        i = counts.get(c, 0)
        refs.append((c, i))
        counts[c] = i + 1
    return counts, refs


IN_COUNTS, IN_REFS = _classes(IN_CHUNKS)
OUT_COUNTS, OUT_REFS = _classes(OUT_CHUNKS)

TRACE = False             # test.py sets this for profiling runs
LAST_EXEC_NS = None       # filled when TRACE

_compiled = None


def _build_graph():
    import concourse.bass as bass
    import concourse.mybir as mybir

    f32 = mybir.dt.float32
    bf16 = mybir.dt.bfloat16

    nc = bass.Bass()

    w_d = nc.declare_dram_parameter("w", [128, SPC * 128], bf16, isOutput=False)
    b_d = nc.declare_dram_parameter("bias", [128, SPC], f32, isOutput=False)
    x_cls = {
        c: nc.declare_dram_parameter(f"x{c}", [n, 128, c * FREE], bf16, isOutput=False)
        for c, n in IN_COUNTS.items()
    }
    o_cls = {
        c: nc.declare_dram_parameter(f"o{c}", [n, 128, c * FREE], bf16, isOutput=True)
        for c, n in OUT_COUNTS.items()
    }

    from contextlib import ExitStack

    with ExitStack() as ctx:
        w_sb = ctx.enter_context(nc.sbuf_tensor([128, SPC * 128], bf16))
        bias_sb = ctx.enter_context(nc.sbuf_tensor([128, SPC], f32))
        in_t = ctx.enter_context(nc.sbuf_tensor([128, IBG * FREE], bf16))
        ot_t = ctx.enter_context(nc.sbuf_tensor([128, OTG * FREE], bf16))
        op = [ctx.enter_context(nc.psum_tensor(f"op{i}", [128, FREE], f32)) for i in range(OB)]
        s_const = ctx.enter_context(nc.semaphore())
        s_mm = ctx.enter_context(nc.semaphore())
        s_add_e = ctx.enter_context(nc.semaphore())
        s_add_o = ctx.enter_context(nc.semaphore())
        s_in = [ctx.enter_context(nc.semaphore(f"s_in{i}")) for i in range(LIN)]
        s_out = [ctx.enter_context(nc.semaphore(f"s_out{i}")) for i in range(LOUT)]
        block = ctx.enter_context(nc.Block())

        def in_sl(g):
            a = (g % IBG) * FREE
            return in_t[:, a : a + FREE]

        def ot_sl(g):
            a = (g % OTG) * FREE
            return ot_t[:, a : a + FREE]

        # out-chunk index containing group g
        def out_chunk_of(g):
            for j, (s, c) in enumerate(zip(OUT_STARTS, OUT_CHUNKS)):
                if s <= g < s + c:
                    return j
            raise AssertionError(g)

        def wait_add(eng, g_prev):
            eng.wait_ge(s_add_e if g_prev % 2 == 0 else s_add_o, g_prev // 2 + 1)

        @block.sync
        def _(sync):
            sync.dma_start(out=w_sb[:], in_=w_d[:]).then_inc(s_const, 16)
            sync.dma_start(out=bias_sb[:], in_=b_d[:]).then_inc(s_const, 16)
            for j, (gs, c) in enumerate(zip(IN_STARTS, IN_CHUNKS)):
                if gs + c > IBG:
                    sync.wait_ge(s_mm, gs + c - IBG)
                (sz, idx) = IN_REFS[j]
                a = (gs % IBG) * FREE
                sync.dma_start(
                    out=in_t[:, a : a + c * FREE], in_=x_cls[sz][idx]
                ).then_inc(s_in[j % LIN], 16)

        @block.tensor
        def _(pe):
            pe.wait_ge(s_const, 32)
            for g in range(G):
                if g in IN_STARTS:
                    j = IN_STARTS.index(g)
                    pe.wait_ge(s_in[j % LIN], 16 * (j // LIN + 1))
                if g >= OB:
                    wait_add(pe, g - OB)
                s = g // GPS
                nc.tensor.matmul(
                    op[g % OB][:],
                    lhsT=w_sb[:, s * 128 : (s + 1) * 128],
                    rhs=in_sl(g),
                    start=True,
                    stop=True,
                ).then_inc(s_mm, 1)

        def add_body(eng, g, emit):
            eng.wait_ge(s_mm, g + 1)
            if g >= OTG:
                dj = out_chunk_of(g - OTG)
                eng.wait_ge(s_out[dj % LOUT], 16 * (dj // LOUT + 1))
            emit(g // GPS)

        @block.vector
        def _(dve):
            dve.wait_ge(s_const, 32)
            for g in range(0, G, 2):
                add_body(
                    dve,
                    g,
                    lambda s, g=g: nc.vector.tensor_scalar_add(
                        ot_sl(g), op[g % OB][:], bias_sb[:, s : s + 1]
                    ).then_inc(s_add_e, 1),
                )

        @block.scalar
        def _(act):
            import concourse.mybir as mybir

            act.wait_ge(s_const, 32)
            for g in range(1, G, 2):
                add_body(
                    act,
                    g,
                    lambda s, g=g: nc.scalar.activation(
                        ot_sl(g),
                        op[g % OB][:],
                        func=mybir.ActivationFunctionType.Identity,
                        bias=bias_sb[:, s : s + 1],
                        scale=1.0,
                    ).then_inc(s_add_o, 1),
                )

        @block.gpsimd
        def _(gp):
            for j, (gs, c) in enumerate(zip(OUT_STARTS, OUT_CHUNKS)):
                ge = gs + c  # one past last group of chunk
                gp.wait_ge(s_add_e, (ge + 1) // 2)
                gp.wait_ge(s_add_o, ge // 2)
                (sz, idx) = OUT_REFS[j]
                a = (gs % OTG) * FREE
                gp.dma_start(
                    out=o_cls[sz][idx], in_=ot_t[:, a : a + c * FREE]
                ).then_inc(s_out[j % LOUT], 16)

    return nc


def _to_bf16(a):
    """f32 contiguous -> bf16 (round-to-nearest-even), fast numpy path."""
    import ml_dtypes

    u = np.ascontiguousarray(a, np.float32).view(np.uint32)
    v = ((u + np.uint32(0x7FFF) + ((u >> np.uint32(16)) & np.uint32(1))) >> np.uint32(16)).astype(
        np.uint16
    )
    return v.view(ml_dtypes.bfloat16)


def _host_factors(x):
    """Per-sample affine factors: kron(I8, vh*std) [128,128] bf16, bias col [128] f32.

    The SVD must run through jax-CPU (jaxlib's LAPACK sgesdd) because the
    reference's output depends on the singular-vector sign conventions of that
    exact implementation.
    """
    import jax
    import jax.numpy as jnp

    cpu = jax.devices("cpu")[0]
    _, svs, vhs = jax.jit(
        lambda a: jnp.linalg.svd(a, full_matrices=False), device=cpu
    )(jax.device_put(x, cpu))
    svs = np.asarray(svs)
    vhs = np.asarray(vhs)

    import ml_dtypes

    ws = np.empty((B, 128, 128), ml_dtypes.bfloat16)
    bs = np.empty((B, 128), np.float32)
    eye8 = np.eye(8, dtype=np.float64)
    for s in range(B):
        Xs = x[s]
        sv, vh = svs[s], vhs[s]
        vh64 = vh.astype(np.float64)
        Mm = vh64 @ vh64
        xbar = Xs.mean(axis=0, dtype=np.float64)
        mean = xbar @ vh64
        e2 = (sv.astype(np.float64) ** 2) @ (Mm**2) / N
        var = np.maximum(e2 - mean**2, 0.0)
        std = np.sqrt(var)
        W = vh64 * std[None, :]
        ws[s] = np.kron(eye8, W).astype(ml_dtypes.bfloat16)
        bs[s] = np.tile(mean, 8).astype(np.float32)
    return ws, bs


def _pack_core(xtb_core):
    """[SPC, 16, N] bf16 -> {size: [n, 128, size*FREE] uint16} per IN_CHUNKS."""
    v = xtb_core.view(np.uint16)
    arrs = {c: np.empty((n, 128, c * FREE), np.uint16) for c, n in IN_COUNTS.items()}
    for j, (gs, c) in enumerate(zip(IN_STARTS, IN_CHUNKS)):
        s, n0 = gs // GPS, (gs % GPS) * (M * FREE)
        seg = v[s][:, n0 : n0 + c * M * FREE]          # [16, c*4096]
        t = seg.reshape(NC, c, M, FREE).transpose(2, 0, 1, 3)  # (m,k,i,f)
        sz, idx = IN_REFS[j]
        arrs[sz][idx] = t.reshape(128, c * FREE)
    return arrs


def _unpack_core(res_core):
    """device outputs -> [SPC, 16, N] uint16 (YT layout)."""
    yt = np.empty((SPC, NC, N), np.uint16)
    for j, (gs, c) in enumerate(zip(OUT_STARTS, OUT_CHUNKS)):
        sz, idx = OUT_REFS[j]
        tile = np.asarray(res_core[f"o{sz}"][idx]).view(np.uint16)  # [128, c*FREE]
        s, n0 = gs // GPS, (gs % GPS) * (M * FREE)
        seg = tile.reshape(M, NC, c, FREE).transpose(1, 2, 0, 3)    # (j,i,m,f)
        yt[s][:, n0 : n0 + c * M * FREE] = seg.reshape(NC, c * M * FREE)
    return yt


def kernel(x):
    global _compiled, LAST_EXEC_NS
    from concourse.bass_utils import run_bass_kernel_spmd

    x = np.ascontiguousarray(np.asarray(x), dtype=np.float32).reshape(B, N, NC)
    ws, bs = _host_factors(x)

    xt = np.ascontiguousarray(x.transpose(0, 2, 1))  # [B, 16, N] f32
    xtb = _to_bf16(xt).reshape(B, NC, N)             # [B, 16, N] bf16

    if _compiled is None:
        _compiled = _build_graph()
    nc = _compiled

    import ml_dtypes

    in_maps = []
    for c in range(CORES):
        s0 = c * SPC
        m = {
            # [128, SPC*128]: sample s's kron at columns s*128..(s+1)*128
            "w": np.ascontiguousarray(
                ws[s0 : s0 + SPC].transpose(1, 0, 2).reshape(128, SPC * 128)
            ),
            "bias": np.ascontiguousarray(bs[s0 : s0 + SPC].T),
        }
        for sz, arr in _pack_core(xtb[s0 : s0 + SPC]).items():
            m[f"x{sz}"] = arr.view(ml_dtypes.bfloat16)
        in_maps.append(m)

    res = run_bass_kernel_spmd(nc, in_maps, core_ids=list(range(CORES)), trace=TRACE)
    LAST_EXEC_NS = res.exec_time_ns

    yt_u = np.empty((B, NC, N), np.uint16)
    for c in range(CORES):
        yt_u[c * SPC : (c + 1) * SPC] = _unpack_core(res.results[c])
    yf = (yt_u.astype(np.uint32) << np.uint32(16)).view(np.float32)  # [B,16,N] f32
    out = np.ascontiguousarray(yf.transpose(0, 2, 1))                # [B,N,16]
    return out.reshape(B, 64, 64, 256)
